# revision 2
# baseline (speedup 1.0000x reference)
"""Trainium2 Bass kernel for nn_Enet_81037442941606 (gnn_message_passing), v2.

Computation (reference):
    g   = enc_out[batch_idx, tgt]                      # [N, D] gather
    h0  = batchnorm(g)  (training stats, biased var)   # [N, D]
    h1  = swish(h0 @ wt2_w.T + wt2_b)                  # [N, C]
    out = h1 @ A.T + h1   (A sparse, NNZ entries)      # [N, C]

v2 strategy (8 cores, tensor parallel over classes):
  * Each core owns 8192 classes; host permutes classes within each
    2048-class quarter so per-(block, src-quarter) edge counts fit the
    chunk caps.
  * h1^T shard computed in bf16 (as v1); additionally quantized to int8
    (q = round(h1 * SCL), |q| <= 127) and exchanged via FOUR 8-way
    AllGathers, one per source quarter, pipelined with the main matmul.
  * spmm gathers (512B int8 rows, 4 SWDGE queues) start as soon as the
    first AllGather lands; parked int8 chunks are upcast to bf16 on DVE
    right before the selection matmuls; sel carries A_values/SCL so no
    dequant pass is needed.  Residual h1T stays exact bf16.
  * One bf16 spill of the half-0 partial into h1T (as v1).
"""

import numpy as np
import ml_dtypes

import concourse.bacc as bacc
import concourse.bass as bass
import concourse.mybir as mybir
import concourse.tile as tile
from concourse import library_config
from concourse.bass_utils import run_bass_kernel_spmd
from concourse.masks import make_identity

B, S, D, C, N = 32, 128, 1024, 65536, 512
NNZ = 262144
EPS = 1e-5
NCORES = 8
CLOC = C // NCORES          # 8192 classes per core
NB = CLOC // 128            # 64 row-blocks per core
ND = D // 128               # 8 contraction chunks
NT = N // 128               # 4 token tiles
P = 128

NQ = 4                      # source quarters = AG chunks = gather levels
TPQ = NB // NQ              # 16 tiles per quarter
QROWS = NCORES * TPQ * P    # 16384 rows per ag quarter tensor
CPQ = CLOC // NQ            # 2048 classes per quarter
WB = 4                      # phase-B block group
GB = 4                      # phase-D block group
NGB = NB // GB

EX_INT8 = True              # exchange h1 as int8 (else bf16)
ZP_MODE = False             # zero-point variant (if HW cast truncates)
SMAX = 6.3
SCL = (120.0 if ZP_MODE else 127.0) / SMAX
ZP_BIAS = 7.5               # added before cast in ZP mode
ZP_EFF = 7.5                # effective zero point folded into corrv
WEX = 256 if EX_INT8 else 512   # exchange row width in bf16 cols

_PROGRAM_CACHE = {}
TRACE = False
TRACE_DIR = None
DEBUG_DUMP = False
LAST_RESULTS = None


def _build_program(prof: tuple):
    """prof = caps[q*NB + b] for q in 0..3: chunks per (src-quarter, block)."""
    prof = prof + (DEBUG_DUMP,)
    if prof in _PROGRAM_CACHE:
        return _PROGRAM_CACHE[prof]
    caps = np.asarray(prof[:NQ * NB], dtype=np.int64).reshape(NQ, NB)
    L = caps.sum(axis=1)                      # chunks per level
    lbase = np.concatenate([[0], np.cumsum(L)])[:NQ]
    off = np.zeros((NQ, NB), dtype=np.int64)  # global chunk offset
    for q in range(NQ):
        off[q] = lbase[q] + np.concatenate([[0], np.cumsum(caps[q])[:-1]])
    tot_ch = int(L.sum())
    maxlevel = int(L.max())

    nc = bacc.Bacc("TRN2", target_bir_lowering=False, debug=False,
                   num_devices=NCORES, num_swdge_queues=4)
    f32 = mybir.dt.float32
    i16 = mybir.dt.int16
    i8 = mybir.dt.int8
    bf16 = mybir.dt.bfloat16

    enc = nc.dram_tensor("enc", [B * S, D], bf16, kind="ExternalInput")
    tgidx = nc.dram_tensor("tgidx", [P, N // 16], i16, kind="ExternalInput")
    wt = nc.dram_tensor("wt", [NB, P, D], bf16, kind="ExternalInput")
    biasv = nc.dram_tensor("biasv", [P, NB], f32, kind="ExternalInput")
    sel = nc.dram_tensor("sel", [P, tot_ch * P], bf16, kind="ExternalInput")
    gidxs = nc.dram_tensor("gidxs", [P, tot_ch * P // 16], i16,
                           kind="ExternalInput")
    if ZP_MODE:
        corrv = nc.dram_tensor("corrv", [P, NB], f32, kind="ExternalInput")
    outT = nc.dram_tensor("outT", [P * NB, N], bf16, kind="ExternalOutput")
    if DEBUG_DUMP:
        dbg_h1 = nc.dram_tensor("dbg_h1", [P, NB * N], bf16,
                                kind="ExternalOutput")
        dbg_ag0 = nc.dram_tensor("dbg_ag0", [QROWS, WEX],
                                 i16 if EX_INT8 else bf16,
                                 kind="ExternalOutput")

    exdt = i16 if EX_INT8 else bf16
    ag_ins = [nc.dram_tensor(f"ag_in{q}", [TPQ * P, WEX], exdt)
              for q in range(NQ)]
    ag_qs = [nc.dram_tensor(f"ag_q{q}", [QROWS, WEX], exdt,
                            addr_space="Shared") for q in range(NQ)]
    # ag_in rows are p-major within the quarter: row = p*TPQ + t
    ag_in_vs = [t.ap().rearrange("(p t) n -> p t n", t=TPQ) for t in ag_ins]
    outT_v = outT.ap().rearrange("(p b) n -> p b n", b=NB)

    qcall = [0]  # global gather-call counter for queue rotation

    with tile.TileContext(nc) as tc:
        with (
            tc.tile_pool(name="persist", bufs=1) as persist,
            tc.tile_pool(name="park", bufs=1) as parkp,
            tc.tile_pool(name="godd", bufs=6) as goddp,
        ):
            h1T = persist.tile([P, NB * N], bf16)
            bias_t = persist.tile([P, NB], f32)
            gidxs_t = persist.tile([P, tot_ch * P // 16], i16)
            ident = persist.tile([P, P], bf16)
            if ZP_MODE:
                corr_t = persist.tile([P, NB], f32)
                nc.sync.dma_start(out=corr_t[:], in_=corrv[:])

            make_identity(nc, ident[:])
            nc.gpsimd.load_library(library_config.mlp)
            nc.sync.dma_start(out=bias_t[:], in_=biasv[:])
            nc.sync.dma_start(out=gidxs_t[:], in_=gidxs[:])
            tgidx_t = persist.tile([P, N // 16], i16)
            nc.sync.dma_start(out=tgidx_t[:], in_=tgidx[:])

            # ---------------- Phases A+B (pools close before phase D) -------
            bpools = (
                tc.tile_pool(name="h0p", bufs=1),
                tc.tile_pool(name="wtp", bufs=2),
                tc.tile_pool(name="stgp", bufs=2),
                tc.tile_pool(name="psB", bufs=4, space="PSUM"),
            )
            h0p, wtp, stgp, psB = [p.__enter__() for p in bpools]
            h0T = h0p.tile([P, ND * N], bf16)

            # ---------------- Phase A: gather + batchnorm + h0^T ------------
            with (
                tc.tile_pool(name="phA", bufs=1) as phA,
                tc.tile_pool(name="psA", bufs=4, space="PSUM") as psA,
            ):
                g_t = phA.tile([P, NT * D], bf16, tag="g")
                nc.gpsimd.dma_gather(
                    g_t[:].rearrange("p (t d) -> p t d", d=D),
                    enc[:], tgidx_t[:], N, N, D)

                sum_s = phA.tile([P, ND], f32, tag="sums")
                sq_s = phA.tile([P, ND], f32, tag="sqs")
                scr = phA.tile([P, N], f32, tag="scr")
                for i in range(ND):
                    for j in range(NT):
                        tp = psA.tile([P, P], bf16, space="PSUM", tag="tp")
                        nc.tensor.transpose(
                            tp[:], g_t[:, j * D + i * P: j * D + (i + 1) * P],
                            ident[:])
                        nc.vector.tensor_copy(
                            out=h0T[:, i * N + j * P: i * N + (j + 1) * P],
                            in_=tp[:])
                    nc.scalar.activation(
                        scr[:], h0T[:, i * N:(i + 1) * N],
                        mybir.ActivationFunctionType.Copy,
                        accum_out=sum_s[:, i:i + 1])
                    nc.scalar.activation(
                        scr[:], h0T[:, i * N:(i + 1) * N],
                        mybir.ActivationFunctionType.Square,
                        accum_out=sq_s[:, i:i + 1])

                mean_s = phA.tile([P, ND], f32, tag="mean")
                rstd_s = phA.tile([P, ND], f32, tag="rstd")
                ex2_s = phA.tile([P, ND], f32, tag="ex2")
                var_s = phA.tile([P, ND], f32, tag="var")
                nc.scalar.mul(mean_s[:], sum_s[:], 1.0 / N)
                nc.scalar.mul(ex2_s[:], sq_s[:], 1.0 / N)
                nc.vector.tensor_tensor(
                    out=var_s[:], in0=mean_s[:], in1=mean_s[:],
                    op=mybir.AluOpType.mult)
                nc.vector.tensor_tensor(
                    out=var_s[:], in0=ex2_s[:], in1=var_s[:],
                    op=mybir.AluOpType.subtract)
                sd_s = phA.tile([P, ND], f32, tag="sd")
                epsb = phA.tile([P, 1], f32, tag="epsb")
                nc.vector.memset(epsb[:], EPS)
                nc.scalar.activation(
                    sd_s[:], var_s[:], mybir.ActivationFunctionType.Sqrt,
                    bias=epsb[:, :1], scale=1.0)
                nc.vector.reciprocal(rstd_s[:], sd_s[:])
                for i in range(ND):
                    nc.vector.tensor_scalar(
                        out=h0T[:, i * N:(i + 1) * N],
                        in0=h0T[:, i * N:(i + 1) * N],
                        scalar1=mean_s[:, i:i + 1],
                        scalar2=rstd_s[:, i:i + 1],
                        op0=mybir.AluOpType.subtract,
                        op1=mybir.AluOpType.mult,
                    )

            # ---------------- Phase B: h1^T = swish(W h0^T + b) -------------
            wt_b = wt.ap().rearrange("(a b) p d -> a b p d", b=WB)
            for a in range(NB // WB):
                wt_a = wtp.tile([P, WB * D], bf16, tag="wt")
                nc.scalar.dma_start(
                    out=wt_a[:].rearrange("p (b d) -> p b d", b=WB),
                    in_=wt_b[a].rearrange("b p d -> p b d"))
                for bsub in range(WB):
                    i = a * WB + bsub
                    h1ps = psB.tile([P, N], f32, space="PSUM", tag="h1ps")
                    for k in range(ND):
                        nc.tensor.matmul(
                            out=h1ps[:],
                            lhsT=wt_a[:, bsub * D + k * P:
                                      bsub * D + (k + 1) * P],
                            rhs=h0T[:, k * N:(k + 1) * N],
                            start=(k == 0), stop=(k == ND - 1),
                        )
                    nc.scalar.activation(
                        h1T[:, i * N:(i + 1) * N], h1ps[:],
                        mybir.ActivationFunctionType.Silu,
                        bias=bias_t[:, i:i + 1], scale=1.0)
                # quantize + stage the 4 finished tiles (p-major rows)
                q = (a * WB) // TPQ
                t0 = (a * WB) % TPQ
                if EX_INT8:
                    stg = stgp.tile([P, WB * N], i8, tag="stg")
                    nc.vector.tensor_scalar(
                        out=stg[:],
                        in0=h1T[:, a * WB * N:(a + 1) * WB * N],
                        scalar1=float(SCL),
                        scalar2=float(ZP_BIAS) if ZP_MODE else 127.0,
                        op0=mybir.AluOpType.mult,
                        op1=(mybir.AluOpType.add if ZP_MODE
                             else mybir.AluOpType.min),
                    )
                    nc.sync.dma_start(
                        out=ag_in_vs[q][:, t0:t0 + WB, :],
                        in_=stg[:].bitcast(i16)
                            .rearrange("p (t n) -> p t n", n=WEX))
                    if ZP_MODE:
                        for bsub in range(WB):
                            i = a * WB + bsub
                            nc.vector.tensor_scalar(
                                out=h1T[:, i * N:(i + 1) * N],
                                in0=h1T[:, i * N:(i + 1) * N],
                                scalar1=corr_t[:, i:i + 1],
                                scalar2=None,
                                op0=mybir.AluOpType.subtract,
                            )
                else:
                    nc.sync.dma_start(
                        out=ag_in_vs[q][:, t0:t0 + WB, :],
                        in_=h1T[:, a * WB * N:(a + 1) * WB * N]
                            .rearrange("p (t n) -> p t n", n=N))
                if t0 + WB == TPQ:
                    nc.gpsimd.collective_compute(
                        "AllGather",
                        mybir.AluOpType.bypass,
                        replica_groups=[list(range(NCORES))],
                        ins=[ag_ins[q][:].opt()],
                        outs=[ag_qs[q][:].opt()],
                    )

            if DEBUG_DUMP:
                nc.sync.dma_start(out=dbg_h1[:], in_=h1T[:])
                ag0v = ag_qs[0].ap().rearrange("(a p) n -> p a n", p=P)
                d0v = dbg_ag0.ap().rearrange("(a p) n -> p a n", p=P)
                with tc.tile_pool(name="dbgp", bufs=2) as dbgp:
                    for i in range(4):
                        dt_ = dbgp.tile([P, 32 * WEX],
                                        i16 if EX_INT8 else bf16, tag="d")
                        nc.sync.dma_start(
                            out=dt_[:].rearrange("p (a n) -> p a n", n=WEX),
                            in_=ag0v[:, i * 32:(i + 1) * 32, :])
                        nc.sync.dma_start(
                            out=d0v[:, i * 32:(i + 1) * 32, :],
                            in_=dt_[:].rearrange("p (a n) -> p a n", n=WEX))

            # ---------------- Phase D: spmm + residual ----------------------
            for p_ in reversed(bpools):
                p_.__exit__(None, None, None)
            with (
                tc.tile_pool(name="selp", bufs=3) as selp,
                tc.tile_pool(name="workp", bufs=4) as workp,
                tc.tile_pool(name="otp", bufs=2) as otp,
                tc.tile_pool(name="psD", bufs=2, space="PSUM") as psD,
            ):
                for h in range(2):
                    qe, qo = 2 * h, 2 * h + 1
                    # even level gathers -> park (early: right after AG_qe)
                    pk = parkp.tile([P, maxlevel * WEX], exdt, tag="park")
                    nlv = int(L[qe])
                    for s0 in range(0, nlv, 8):
                        ks = min(8, nlv - s0)
                        nc.gpsimd.dma_gather(
                            pk[:, s0 * WEX:(s0 + ks) * WEX]
                                .rearrange("p (c n) -> p c n", n=WEX),
                            ag_qs[qe][:],
                            gidxs_t[:, (lbase[qe] + s0) * 8:
                                    (lbase[qe] + s0 + ks) * 8],
                            ks * P, ks * P, WEX,
                            queue_num=qcall[0] % 2)
                        qcall[0] += 1
                    # odd level gathered per processing group
                    for gb in range(NGB):
                        b0 = gb * GB
                        k0 = int(caps[qe][b0:b0 + GB].sum())
                        k1 = int(caps[qo][b0:b0 + GB].sum())
                        g_o = goddp.tile([P, 8 * WEX], exdt, tag="go")
                        nc.gpsimd.dma_gather(
                            g_o[:, :k1 * WEX]
                                .rearrange("p (c n) -> p c n", n=WEX),
                            ag_qs[qo][:],
                            gidxs_t[:, off[qo][b0] * 8:
                                    (off[qo][b0] + k1) * 8],
                            k1 * P, k1 * P, WEX,
                            queue_num=2 + qcall[0] % 2)
                        qcall[0] += 1
                        sel_t = selp.tile([P, 16 * P], bf16, tag="sel")
                        nc.sync.dma_start(
                            out=sel_t[:, :k0 * P],
                            in_=sel[:, off[qe][b0] * P:
                                    (off[qe][b0] + k0) * P])
                        nc.sync.dma_start(
                            out=sel_t[:, k0 * P:(k0 + k1) * P],
                            in_=sel[:, off[qo][b0] * P:
                                    (off[qo][b0] + k1) * P])
                        if h == 1:
                            o_t = otp.tile([P, GB * N], bf16, tag="ot")
                        for bi in range(GB):
                            b = b0 + bi
                            ce = int(caps[qe][b])
                            co = int(caps[qo][b])
                            nch = ce + co
                            je = int(off[qe][b] - lbase[qe])
                            jo = int(off[qo][b] - off[qo][b0])
                            if EX_INT8:
                                w_t = workp.tile([P, 4 * N], bf16, tag="wk")
                                use_act = (b % 2 == 1)
                                for (src, lo, cnt, dst0) in (
                                        (pk, je, ce, 0),
                                        (g_o, jo, co, ce)):
                                    if cnt == 0:
                                        continue
                                    sap = src[:, lo * WEX:
                                              (lo + cnt) * WEX].bitcast(i8)
                                    dap = w_t[:, dst0 * N:(dst0 + cnt) * N]
                                    if use_act:
                                        nc.scalar.activation(
                                            dap, sap,
                                            mybir.ActivationFunctionType.Copy)
                                    else:
                                        nc.vector.tensor_copy(
                                            out=dap, in_=sap)
                            acc = psD.tile([P, N], f32, space="PSUM",
                                           tag=f"acc{bi}")
                            sle = int(off[qe][b] - off[qe][b0])
                            slo = k0 + jo
                            for s in range(nch):
                                if s < ce:
                                    lcol = (sle + s) * P
                                else:
                                    lcol = (slo + (s - ce)) * P
                                if EX_INT8:
                                    rr = w_t[:, s * N:(s + 1) * N]
                                elif s < ce:
                                    rr = pk[:, (je + s) * N:(je + s + 1) * N]
                                else:
                                    rr = g_o[:, (jo + s - ce) * N:
                                             (jo + s - ce + 1) * N]
                                nc.tensor.matmul(
                                    out=acc[:],
                                    lhsT=sel_t[:, lcol:lcol + P],
                                    rhs=rr,
                                    start=(s == 0), stop=(s == nch - 1),
                                )
                            if h == 0:
                                nc.vector.tensor_tensor(
                                    out=h1T[:, b * N:(b + 1) * N],
                                    in0=acc[:],
                                    in1=h1T[:, b * N:(b + 1) * N],
                                    op=mybir.AluOpType.add)
                            else:
                                nc.vector.tensor_tensor(
                                    out=o_t[:, bi * N:(bi + 1) * N],
                                    in0=acc[:],
                                    in1=h1T[:, b * N:(b + 1) * N],
                                    op=mybir.AluOpType.add)
                        if h == 1:
                            nc.sync.dma_start(
                                out=outT_v[:, b0:b0 + GB, :],
                                in_=o_t[:].rearrange("p (b n) -> p b n", n=N))

    nc.compile()
    _PROGRAM_CACHE[prof] = nc
    return nc


def _wrap_idx16(flat_idx):
    n = len(flat_idx)
    assert n % 16 == 0
    a = np.asarray(flat_idx, dtype=np.int64)
    assert (a >= 0).all() and (a < 32768).all()
    w = a.reshape(n // 16, 16).T.astype(np.int16)
    return np.ascontiguousarray(np.tile(w, (8, 1)))


def _pack_rank4(degs, caps):
    """Assign CLOC local classes to blocks, quarter-preserving, balancing the
    per-(block, src-quarter) degree under caps[q][b]*P."""
    old2new = np.empty(CLOC, dtype=np.int64)
    for Q in range(NQ):
        lo = Q * CPQ
        cls = np.arange(lo, lo + CPQ)
        d = degs[:, cls].astype(np.float64)          # [4, CPQ]
        order = np.argsort(-d.sum(0), kind="stable")
        blocks = np.arange(Q * TPQ, (Q + 1) * TPQ)
        cap = (caps[:, blocks] * P).astype(np.float64)
        load = np.zeros((NQ, TPQ))
        cnt = np.zeros(TPQ, dtype=np.int64)
        for idx in order:
            dv = d[:, idx:idx + 1]
            score = ((load + dv) / cap).max(axis=0)
            bad = (cnt >= P) | ((load + dv) > cap).any(axis=0)
            score[bad] = np.inf
            bb = int(np.argmin(score))
            if not np.isfinite(score[bb]):
                return None
            old2new[cls[idx]] = blocks[bb] * P + cnt[bb]
            load[:, bb] += dv[:, 0]
            cnt[bb] += 1
    return old2new


def _prep_host(enc_out, wt2_w, wt2_b, A_values, batch_idx, tgt, A_indices):
    EX_NP = ml_dtypes.bfloat16
    enc_flat = np.ascontiguousarray(
        np.asarray(enc_out, dtype=np.float32).reshape(B * S, D)
    ).astype(ml_dtypes.bfloat16)
    flat_idx = (np.asarray(batch_idx, dtype=np.int64) * S
                + np.asarray(tgt, dtype=np.int64))
    tgidx_host = _wrap_idx16(flat_idx)

    wt2_w = np.asarray(wt2_w, dtype=np.float32)
    wt2_b = np.asarray(wt2_b, dtype=np.float32)
    rows_all = np.asarray(A_indices[0], dtype=np.int64)
    cols_all = np.asarray(A_indices[1], dtype=np.int64)
    vals_all = np.asarray(A_values, dtype=np.float32)

    # src quarter of an edge is invariant under the quarter-preserving perm
    qsrc_all = (cols_all % CLOC) // CPQ

    rank_data = []
    for r in range(NCORES):
        m = (rows_all // CLOC) == r
        rl = (rows_all[m] - r * CLOC).astype(np.int64)
        cc = cols_all[m]
        vv = vals_all[m]
        qs = qsrc_all[m]
        degs = np.stack([np.bincount(rl[qs == q], minlength=CLOC)
                         for q in range(NQ)])
        rank_data.append((rl, cc, vv, qs, degs))

    nfat = 2
    while True:
        caps = np.ones((NQ, NB), dtype=np.int64)
        for Q in range(NQ):
            caps[:, (Q + 1) * TPQ - nfat:(Q + 1) * TPQ] = 2
        perms = []
        ok = True
        for r in range(NCORES):
            p_ = _pack_rank4(rank_data[r][4], caps)
            if p_ is None:
                ok = False
                break
            perms.append(p_)
        if ok:
            break
        nfat += 2
        if nfat > TPQ:
            raise RuntimeError("packing failed")

    prof = tuple(int(x) for x in caps.reshape(-1))
    L = caps.sum(axis=1)
    lbase = np.concatenate([[0], np.cumsum(L)])[:NQ]
    off = np.zeros((NQ, NB), dtype=np.int64)
    for q in range(NQ):
        off[q] = lbase[q] + np.concatenate([[0], np.cumsum(caps[q])[:-1]])
    tot_ch = int(L.sum())
    new2old = [np.argsort(p_) for p_ in perms]

    selscale = 1.0 / SCL if EX_INT8 else 1.0

    per_rank = []
    for r in range(NCORES):
        rl, cc, vv, qs, _ = rank_data[r]
        rl_new = perms[r][rl]
        blk = rl_new // P
        clsmod = rl_new % P

        # gathered source row, relative to its quarter's ag tensor
        r2 = cc // CLOC
        lnew = np.empty(len(cc), dtype=np.int64)
        for r3 in range(NCORES):
            m3 = r2 == r3
            lnew[m3] = perms[r3][cc[m3] % CLOC]
        t2 = lnew // P
        assert (t2 // TPQ == qs).all()
        tq = t2 % TPQ
        p2 = lnew % P
        rowh = r2 * (TPQ * P) + p2 * TPQ + tq

        sel_host = np.zeros((P, tot_ch * P), dtype=EX_NP)
        gidx_flat = np.zeros(tot_ch * P, dtype=np.int64)
        orderkey = np.lexsort((rowh, blk + NB * qs))
        bl_s = blk[orderkey]
        qs_s = qs[orderkey]
        vv_s = vv[orderkey]
        cm_s = clsmod[orderkey]
        rh_s = rowh[orderkey]
        key = qs_s * NB + bl_s
        counts = np.bincount(key, minlength=NQ * NB)
        starts = np.zeros(NQ * NB, dtype=np.int64)
        starts[1:] = np.cumsum(counts)[:-1]
        pos = np.arange(len(key)) - starts[key]
        chlim = caps.reshape(-1)
        assert (pos < chlim[key] * P).all(), "chunk capacity overflow"
        gch = off[qs_s, bl_s] + pos // P
        slot = pos % P
        sel_host[slot, gch * P + cm_s] = (vv_s * selscale).astype(EX_NP)
        gidx_flat[gch * P + slot] = rh_s
        gidxs_host = _wrap_idx16(gidx_flat)

        rows = slice(r * CLOC, (r + 1) * CLOC)
        wr = wt2_w[rows][new2old[r]]
        wt_host = np.ascontiguousarray(
            wr.reshape(NB, P, ND, P).transpose(0, 3, 2, 1)
        ).reshape(NB, P, D).astype(ml_dtypes.bfloat16)
        bias_host = np.ascontiguousarray(
            wt2_b[rows][new2old[r]].reshape(NB, P).T)
        rd = {
            "enc": enc_flat,
            "tgidx": tgidx_host,
            "wt": wt_host,
            "biasv": bias_host,
            "sel": sel_host,
            "gidxs": gidxs_host,
        }
        if ZP_MODE:
            selv = (vv * selscale).astype(EX_NP).astype(np.float64)
            rowsum = np.bincount(rl_new, weights=selv, minlength=CLOC)
            corr = (ZP_EFF * rowsum).astype(np.float32)
            rd["corrv"] = np.ascontiguousarray(corr.reshape(NB, P).T)
        per_rank.append(rd)
    return per_rank, prof, new2old


def kernel(**inputs) -> np.ndarray:
    per_rank, prof, new2old = _prep_host(
        inputs["enc_out"], inputs["wt2_w"], inputs["wt2_b"],
        inputs["A_values"], inputs["batch_idx"], inputs["tgt"],
        inputs["A_indices"])
    nc = _build_program(prof)
    res = None
    last_exc = None
    for _attempt in range(3):
        try:
            res = run_bass_kernel_spmd(
                nc, per_rank, core_ids=list(range(NCORES)), trace=TRACE,
                tmpdir=TRACE_DIR)
            break
        except Exception as e:
            last_exc = e
    if res is None:
        raise last_exc
    global LAST_RESULTS
    LAST_RESULTS = res
    outT_full = np.empty((C, N), dtype=np.float32)
    for r in range(NCORES):
        arr = np.asarray(res.results[r]["outT"])
        arr = arr.reshape(P, NB, N).transpose(1, 0, 2).reshape(CLOC, N)
        outT_full[r * CLOC + new2old[r]] = arr.astype(np.float32)
    return np.ascontiguousarray(outT_full.T)


# revision 3
# speedup vs baseline: 1.0218x; 1.0218x over previous
"""Trainium2 Bass kernel for nn_Enet_81037442941606 (gnn_message_passing), v2.

Computation (reference):
    g   = enc_out[batch_idx, tgt]                      # [N, D] gather
    h0  = batchnorm(g)  (training stats, biased var)   # [N, D]
    h1  = swish(h0 @ wt2_w.T + wt2_b)                  # [N, C]
    out = h1 @ A.T + h1   (A sparse, NNZ entries)      # [N, C]

v2 strategy (8 cores, tensor parallel over classes):
  * Each core owns 8192 classes; host permutes classes within each
    2048-class quarter so per-(block, src-quarter) edge counts fit the
    chunk caps.
  * h1^T shard computed in bf16 (as v1); additionally quantized to int8
    (q = round(h1 * SCL), |q| <= 127) and exchanged via FOUR 8-way
    AllGathers, one per source quarter, pipelined with the main matmul.
  * spmm gathers (512B int8 rows, 4 SWDGE queues) start as soon as the
    first AllGather lands; parked int8 chunks are upcast to bf16 on DVE
    right before the selection matmuls; sel carries A_values/SCL so no
    dequant pass is needed.  Residual h1T stays exact bf16.
  * One bf16 spill of the half-0 partial into h1T (as v1).
"""

import numpy as np
import ml_dtypes

import concourse.bacc as bacc
import concourse.bass as bass
import concourse.mybir as mybir
import concourse.tile as tile
from concourse import library_config
from concourse.bass_utils import run_bass_kernel_spmd
from concourse.masks import make_identity

B, S, D, C, N = 32, 128, 1024, 65536, 512
NNZ = 262144
EPS = 1e-5
NCORES = 8
CLOC = C // NCORES          # 8192 classes per core
NB = CLOC // 128            # 64 row-blocks per core
ND = D // 128               # 8 contraction chunks
NT = N // 128               # 4 token tiles
P = 128

NQ = 4                      # source quarters = AG chunks = gather levels
TPQ = NB // NQ              # 16 tiles per quarter
QROWS = NCORES * TPQ * P    # 16384 rows per ag quarter tensor
CPQ = CLOC // NQ            # 2048 classes per quarter
WB = 4                      # phase-B block group
GB = 4                      # phase-D block group
NGB = NB // GB

EX_INT8 = True              # exchange h1 as int8 (else bf16)
ZP_MODE = False             # zero-point variant (if HW cast truncates)
SMAX = 6.3
SCL = (120.0 if ZP_MODE else 127.0) / SMAX
ZP_BIAS = 7.5               # added before cast in ZP mode
ZP_EFF = 7.5                # effective zero point folded into corrv
WEX = 256 if EX_INT8 else 512   # exchange row width in bf16 cols

_PROGRAM_CACHE = {}
TRACE = False
TRACE_DIR = None
DEBUG_DUMP = False
LAST_RESULTS = None


def _build_program(prof: tuple):
    """prof = caps[q*NB + b] for q in 0..3: chunks per (src-quarter, block)."""
    prof = prof + (DEBUG_DUMP,)
    if prof in _PROGRAM_CACHE:
        return _PROGRAM_CACHE[prof]
    caps = np.asarray(prof[:NQ * NB], dtype=np.int64).reshape(NQ, NB)
    L = caps.sum(axis=1)                      # chunks per level
    lbase = np.concatenate([[0], np.cumsum(L)])[:NQ]
    off = np.zeros((NQ, NB), dtype=np.int64)  # global chunk offset
    for q in range(NQ):
        off[q] = lbase[q] + np.concatenate([[0], np.cumsum(caps[q])[:-1]])
    tot_ch = int(L.sum())
    maxlevel = int(L.max())

    nc = bacc.Bacc("TRN2", target_bir_lowering=False, debug=False,
                   num_devices=NCORES, num_swdge_queues=4)
    f32 = mybir.dt.float32
    i16 = mybir.dt.int16
    i8 = mybir.dt.int8
    bf16 = mybir.dt.bfloat16

    enc = nc.dram_tensor("enc", [B * S, D], bf16, kind="ExternalInput")
    tgidx = nc.dram_tensor("tgidx", [P, N // 16], i16, kind="ExternalInput")
    wt = nc.dram_tensor("wt", [NB, P, D], bf16, kind="ExternalInput")
    biasv = nc.dram_tensor("biasv", [P, NB], f32, kind="ExternalInput")
    sel = nc.dram_tensor("sel", [P, tot_ch * P], bf16, kind="ExternalInput")
    gidxs = nc.dram_tensor("gidxs", [P, tot_ch * P // 16], i16,
                           kind="ExternalInput")
    if ZP_MODE:
        corrv = nc.dram_tensor("corrv", [P, NB], f32, kind="ExternalInput")
    outT = nc.dram_tensor("outT", [P * NB, N], bf16, kind="ExternalOutput")
    if DEBUG_DUMP:
        dbg_h1 = nc.dram_tensor("dbg_h1", [P, NB * N], bf16,
                                kind="ExternalOutput")
        dbg_ag0 = nc.dram_tensor("dbg_ag0", [QROWS, WEX],
                                 i16 if EX_INT8 else bf16,
                                 kind="ExternalOutput")

    exdt = i16 if EX_INT8 else bf16
    ag_ins = [nc.dram_tensor(f"ag_in{q}", [TPQ * P, WEX], exdt)
              for q in range(NQ)]
    ag_qs = [nc.dram_tensor(f"ag_q{q}", [QROWS, WEX], exdt,
                            addr_space="Shared") for q in range(NQ)]
    # ag_in rows are p-major within the quarter: row = p*TPQ + t
    ag_in_vs = [t.ap().rearrange("(p t) n -> p t n", t=TPQ) for t in ag_ins]
    outT_v = outT.ap().rearrange("(p b) n -> p b n", b=NB)

    qcall = [0]  # global gather-call counter for queue rotation

    with tile.TileContext(nc) as tc:
        with (
            tc.tile_pool(name="persist", bufs=1) as persist,
            tc.tile_pool(name="park", bufs=1) as parkp,
            tc.tile_pool(name="godd", bufs=6) as goddp,
        ):
            h1T = persist.tile([P, NB * N], bf16)
            bias_t = persist.tile([P, NB], f32)
            gidxs_t = persist.tile([P, tot_ch * P // 16], i16)
            ident = persist.tile([P, P], bf16)
            if ZP_MODE:
                corr_t = persist.tile([P, NB], f32)
                nc.sync.dma_start(out=corr_t[:], in_=corrv[:])

            make_identity(nc, ident[:])
            nc.gpsimd.load_library(library_config.mlp)
            nc.sync.dma_start(out=bias_t[:], in_=biasv[:])
            nc.sync.dma_start(out=gidxs_t[:], in_=gidxs[:])
            tgidx_t = persist.tile([P, N // 16], i16)
            nc.sync.dma_start(out=tgidx_t[:], in_=tgidx[:])

            # ---------------- Phases A+B (pools close before phase D) -------
            bpools = (
                tc.tile_pool(name="h0p", bufs=1),
                tc.tile_pool(name="wtp", bufs=3),
                tc.tile_pool(name="stgp", bufs=2),
                tc.tile_pool(name="psB", bufs=6, space="PSUM"),
            )
            h0p, wtp, stgp, psB = [p.__enter__() for p in bpools]
            h0T = h0p.tile([P, ND * N], bf16)

            # ---------------- Phase A: gather + batchnorm + h0^T ------------
            with (
                tc.tile_pool(name="phA", bufs=1) as phA,
                tc.tile_pool(name="psA", bufs=2, space="PSUM") as psA,
            ):
                g_t = phA.tile([P, NT * D], bf16, tag="g")
                nc.gpsimd.dma_gather(
                    g_t[:].rearrange("p (t d) -> p t d", d=D),
                    enc[:], tgidx_t[:], N, N, D)

                sum_s = phA.tile([P, ND], f32, tag="sums")
                sq_s = phA.tile([P, ND], f32, tag="sqs")
                scr = phA.tile([P, N], f32, tag="scr")
                for i in range(ND):
                    for j in range(NT):
                        tp = psA.tile([P, P], bf16, space="PSUM", tag="tp")
                        nc.tensor.transpose(
                            tp[:], g_t[:, j * D + i * P: j * D + (i + 1) * P],
                            ident[:])
                        nc.vector.tensor_copy(
                            out=h0T[:, i * N + j * P: i * N + (j + 1) * P],
                            in_=tp[:])
                    nc.scalar.activation(
                        scr[:], h0T[:, i * N:(i + 1) * N],
                        mybir.ActivationFunctionType.Copy,
                        accum_out=sum_s[:, i:i + 1])
                    nc.scalar.activation(
                        scr[:], h0T[:, i * N:(i + 1) * N],
                        mybir.ActivationFunctionType.Square,
                        accum_out=sq_s[:, i:i + 1])

                mean_s = phA.tile([P, ND], f32, tag="mean")
                rstd_s = phA.tile([P, ND], f32, tag="rstd")
                ex2_s = phA.tile([P, ND], f32, tag="ex2")
                var_s = phA.tile([P, ND], f32, tag="var")
                nc.scalar.mul(mean_s[:], sum_s[:], 1.0 / N)
                nc.scalar.mul(ex2_s[:], sq_s[:], 1.0 / N)
                nc.vector.tensor_tensor(
                    out=var_s[:], in0=mean_s[:], in1=mean_s[:],
                    op=mybir.AluOpType.mult)
                nc.vector.tensor_tensor(
                    out=var_s[:], in0=ex2_s[:], in1=var_s[:],
                    op=mybir.AluOpType.subtract)
                sd_s = phA.tile([P, ND], f32, tag="sd")
                epsb = phA.tile([P, 1], f32, tag="epsb")
                nc.vector.memset(epsb[:], EPS)
                nc.scalar.activation(
                    sd_s[:], var_s[:], mybir.ActivationFunctionType.Sqrt,
                    bias=epsb[:, :1], scale=1.0)
                nc.vector.reciprocal(rstd_s[:], sd_s[:])
                for i in range(ND):
                    nc.vector.tensor_scalar(
                        out=h0T[:, i * N:(i + 1) * N],
                        in0=h0T[:, i * N:(i + 1) * N],
                        scalar1=mean_s[:, i:i + 1],
                        scalar2=rstd_s[:, i:i + 1],
                        op0=mybir.AluOpType.subtract,
                        op1=mybir.AluOpType.mult,
                    )

            # ---------------- Phase B: h1^T = swish(W h0^T + b) -------------
            wt_b = wt.ap().rearrange("(a b) p d -> a b p d", b=WB)
            for a in range(NB // WB):
                wt_a = wtp.tile([P, WB * D], bf16, tag="wt")
                nc.scalar.dma_start(
                    out=wt_a[:].rearrange("p (b d) -> p b d", b=WB),
                    in_=wt_b[a].rearrange("b p d -> p b d"))
                for bsub in range(WB):
                    i = a * WB + bsub
                    h1ps = psB.tile([P, N], f32, space="PSUM", tag="h1ps")
                    for k in range(ND):
                        nc.tensor.matmul(
                            out=h1ps[:],
                            lhsT=wt_a[:, bsub * D + k * P:
                                      bsub * D + (k + 1) * P],
                            rhs=h0T[:, k * N:(k + 1) * N],
                            start=(k == 0), stop=(k == ND - 1),
                        )
                    nc.scalar.activation(
                        h1T[:, i * N:(i + 1) * N], h1ps[:],
                        mybir.ActivationFunctionType.Silu,
                        bias=bias_t[:, i:i + 1], scale=1.0)
                # quantize + stage the 4 finished tiles (p-major rows)
                q = (a * WB) // TPQ
                t0 = (a * WB) % TPQ
                if EX_INT8:
                    stg = stgp.tile([P, WB * N], i8, tag="stg")
                    nc.vector.tensor_scalar(
                        out=stg[:],
                        in0=h1T[:, a * WB * N:(a + 1) * WB * N],
                        scalar1=float(SCL),
                        scalar2=float(ZP_BIAS) if ZP_MODE else 127.0,
                        op0=mybir.AluOpType.mult,
                        op1=(mybir.AluOpType.add if ZP_MODE
                             else mybir.AluOpType.min),
                    )
                    nc.sync.dma_start(
                        out=ag_in_vs[q][:, t0:t0 + WB, :],
                        in_=stg[:].bitcast(i16)
                            .rearrange("p (t n) -> p t n", n=WEX))
                    if ZP_MODE:
                        for bsub in range(WB):
                            i = a * WB + bsub
                            nc.vector.tensor_scalar(
                                out=h1T[:, i * N:(i + 1) * N],
                                in0=h1T[:, i * N:(i + 1) * N],
                                scalar1=corr_t[:, i:i + 1],
                                scalar2=None,
                                op0=mybir.AluOpType.subtract,
                            )
                else:
                    nc.sync.dma_start(
                        out=ag_in_vs[q][:, t0:t0 + WB, :],
                        in_=h1T[:, a * WB * N:(a + 1) * WB * N]
                            .rearrange("p (t n) -> p t n", n=N))
                if t0 + WB == TPQ:
                    nc.gpsimd.collective_compute(
                        "AllGather",
                        mybir.AluOpType.bypass,
                        replica_groups=[list(range(NCORES))],
                        ins=[ag_ins[q][:].opt()],
                        outs=[ag_qs[q][:].opt()],
                    )

            if DEBUG_DUMP:
                nc.sync.dma_start(out=dbg_h1[:], in_=h1T[:])
                ag0v = ag_qs[0].ap().rearrange("(a p) n -> p a n", p=P)
                d0v = dbg_ag0.ap().rearrange("(a p) n -> p a n", p=P)
                with tc.tile_pool(name="dbgp", bufs=2) as dbgp:
                    for i in range(4):
                        dt_ = dbgp.tile([P, 32 * WEX],
                                        i16 if EX_INT8 else bf16, tag="d")
                        nc.sync.dma_start(
                            out=dt_[:].rearrange("p (a n) -> p a n", n=WEX),
                            in_=ag0v[:, i * 32:(i + 1) * 32, :])
                        nc.sync.dma_start(
                            out=d0v[:, i * 32:(i + 1) * 32, :],
                            in_=dt_[:].rearrange("p (a n) -> p a n", n=WEX))

            # ---------------- Phase D: spmm + residual ----------------------
            for p_ in reversed(bpools):
                p_.__exit__(None, None, None)
            with (
                tc.tile_pool(name="selp", bufs=3) as selp,
                tc.tile_pool(name="workp", bufs=4) as workp,
                tc.tile_pool(name="otp", bufs=2) as otp,
                tc.tile_pool(name="psD", bufs=2, space="PSUM") as psD,
            ):
                for h in range(2):
                    qe, qo = 2 * h, 2 * h + 1
                    # even level gathers -> park (early: right after AG_qe)
                    pk = parkp.tile([P, maxlevel * WEX], exdt, tag="park")
                    nlv = int(L[qe])
                    for s0 in range(0, nlv, 8):
                        ks = min(8, nlv - s0)
                        nc.gpsimd.dma_gather(
                            pk[:, s0 * WEX:(s0 + ks) * WEX]
                                .rearrange("p (c n) -> p c n", n=WEX),
                            ag_qs[qe][:],
                            gidxs_t[:, (lbase[qe] + s0) * 8:
                                    (lbase[qe] + s0 + ks) * 8],
                            ks * P, ks * P, WEX,
                            queue_num=qcall[0] % 2)
                        qcall[0] += 1
                    # odd level: batched 8-chunk gathers into rotating tiles
                    nodd = int(L[qo])
                    otiles = []
                    for s0 in range(0, nodd, 8):
                        ks = min(8, nodd - s0)
                        g_o = goddp.tile([P, 8 * WEX], exdt, tag="go")
                        nc.gpsimd.dma_gather(
                            g_o[:, :ks * WEX]
                                .rearrange("p (c n) -> p c n", n=WEX),
                            ag_qs[qo][:],
                            gidxs_t[:, (lbase[qo] + s0) * 8:
                                    (lbase[qo] + s0 + ks) * 8],
                            ks * P, ks * P, WEX,
                            queue_num=2 + qcall[0] % 2)
                        qcall[0] += 1
                        otiles.append(g_o)
                    for gb in range(NGB):
                        b0 = gb * GB
                        k0 = int(caps[qe][b0:b0 + GB].sum())
                        k1 = int(caps[qo][b0:b0 + GB].sum())
                        sel_t = selp.tile([P, 16 * P], bf16, tag="sel")
                        nc.sync.dma_start(
                            out=sel_t[:, :k0 * P],
                            in_=sel[:, off[qe][b0] * P:
                                    (off[qe][b0] + k0) * P])
                        nc.sync.dma_start(
                            out=sel_t[:, k0 * P:(k0 + k1) * P],
                            in_=sel[:, off[qo][b0] * P:
                                    (off[qo][b0] + k1) * P])
                        if h == 1:
                            o_t = otp.tile([P, GB * N], bf16, tag="ot")
                        for bi in range(GB):
                            b = b0 + bi
                            ce = int(caps[qe][b])
                            co = int(caps[qo][b])
                            nch = ce + co
                            je = int(off[qe][b] - lbase[qe])
                            jol = int(off[qo][b] - lbase[qo])
                            if EX_INT8:
                                w_t = workp.tile([P, 4 * N], bf16, tag="wk")
                                use_act = (b % 2 == 1)
                                segs = []
                                if ce:
                                    segs.append((pk, je, ce, 0))
                                for sj in range(co):
                                    j = jol + sj
                                    segs.append((otiles[j // 8], j % 8, 1,
                                                 ce + sj))
                                # merge adjacent odd chunks in the same tile
                                merged = []
                                for seg in segs:
                                    if (merged
                                            and merged[-1][0] is seg[0]
                                            and merged[-1][1] + merged[-1][2]
                                            == seg[1]):
                                        m = merged.pop()
                                        merged.append((m[0], m[1],
                                                       m[2] + seg[2], m[3]))
                                    else:
                                        merged.append(seg)
                                for (src, lo, cnt, dst0) in merged:
                                    sap = src[:, lo * WEX:
                                              (lo + cnt) * WEX].bitcast(i8)
                                    dap = w_t[:, dst0 * N:(dst0 + cnt) * N]
                                    if use_act:
                                        nc.scalar.activation(
                                            dap, sap,
                                            mybir.ActivationFunctionType.Copy)
                                    else:
                                        nc.vector.tensor_copy(
                                            out=dap, in_=sap)
                            acc = psD.tile([P, N], f32, space="PSUM",
                                           tag=f"acc{bi}")
                            sle = int(off[qe][b] - off[qe][b0])
                            slo = k0 + int(off[qo][b] - off[qo][b0])
                            for s in range(nch):
                                if s < ce:
                                    lcol = (sle + s) * P
                                else:
                                    lcol = (slo + (s - ce)) * P
                                if EX_INT8:
                                    rr = w_t[:, s * N:(s + 1) * N]
                                elif s < ce:
                                    rr = pk[:, (je + s) * N:(je + s + 1) * N]
                                else:
                                    j = jol + s - ce
                                    rr = otiles[j // 8][:, (j % 8) * N:
                                                       (j % 8 + 1) * N]
                                nc.tensor.matmul(
                                    out=acc[:],
                                    lhsT=sel_t[:, lcol:lcol + P],
                                    rhs=rr,
                                    start=(s == 0), stop=(s == nch - 1),
                                )
                            if h == 0:
                                nc.vector.tensor_tensor(
                                    out=h1T[:, b * N:(b + 1) * N],
                                    in0=acc[:],
                                    in1=h1T[:, b * N:(b + 1) * N],
                                    op=mybir.AluOpType.add)
                            else:
                                nc.vector.tensor_tensor(
                                    out=o_t[:, bi * N:(bi + 1) * N],
                                    in0=acc[:],
                                    in1=h1T[:, b * N:(b + 1) * N],
                                    op=mybir.AluOpType.add)
                        if h == 1:
                            nc.sync.dma_start(
                                out=outT_v[:, b0:b0 + GB, :],
                                in_=o_t[:].rearrange("p (b n) -> p b n", n=N))

    nc.compile()
    _PROGRAM_CACHE[prof] = nc
    return nc


def _wrap_idx16(flat_idx):
    n = len(flat_idx)
    assert n % 16 == 0
    a = np.asarray(flat_idx, dtype=np.int64)
    assert (a >= 0).all() and (a < 32768).all()
    w = a.reshape(n // 16, 16).T.astype(np.int16)
    return np.ascontiguousarray(np.tile(w, (8, 1)))


def _pack_rank4(degs, caps):
    """Assign CLOC local classes to blocks, quarter-preserving, balancing the
    per-(block, src-quarter) degree under caps[q][b]*P."""
    old2new = np.empty(CLOC, dtype=np.int64)
    for Q in range(NQ):
        lo = Q * CPQ
        cls = np.arange(lo, lo + CPQ)
        d = degs[:, cls].astype(np.float64)          # [4, CPQ]
        order = np.argsort(-d.sum(0), kind="stable")
        blocks = np.arange(Q * TPQ, (Q + 1) * TPQ)
        cap = (caps[:, blocks] * P).astype(np.float64)
        load = np.zeros((NQ, TPQ))
        cnt = np.zeros(TPQ, dtype=np.int64)
        for idx in order:
            dv = d[:, idx:idx + 1]
            score = ((load + dv) / cap).max(axis=0)
            bad = (cnt >= P) | ((load + dv) > cap).any(axis=0)
            score[bad] = np.inf
            bb = int(np.argmin(score))
            if not np.isfinite(score[bb]):
                return None
            old2new[cls[idx]] = blocks[bb] * P + cnt[bb]
            load[:, bb] += dv[:, 0]
            cnt[bb] += 1
    return old2new


def _prep_host(enc_out, wt2_w, wt2_b, A_values, batch_idx, tgt, A_indices):
    EX_NP = ml_dtypes.bfloat16
    enc_flat = np.ascontiguousarray(
        np.asarray(enc_out, dtype=np.float32).reshape(B * S, D)
    ).astype(ml_dtypes.bfloat16)
    flat_idx = (np.asarray(batch_idx, dtype=np.int64) * S
                + np.asarray(tgt, dtype=np.int64))
    tgidx_host = _wrap_idx16(flat_idx)

    wt2_w = np.asarray(wt2_w, dtype=np.float32)
    wt2_b = np.asarray(wt2_b, dtype=np.float32)
    rows_all = np.asarray(A_indices[0], dtype=np.int64)
    cols_all = np.asarray(A_indices[1], dtype=np.int64)
    vals_all = np.asarray(A_values, dtype=np.float32)

    # src quarter of an edge is invariant under the quarter-preserving perm
    qsrc_all = (cols_all % CLOC) // CPQ

    rank_data = []
    for r in range(NCORES):
        m = (rows_all // CLOC) == r
        rl = (rows_all[m] - r * CLOC).astype(np.int64)
        cc = cols_all[m]
        vv = vals_all[m]
        qs = qsrc_all[m]
        degs = np.stack([np.bincount(rl[qs == q], minlength=CLOC)
                         for q in range(NQ)])
        rank_data.append((rl, cc, vv, qs, degs))

    nfat = 2
    while True:
        caps = np.ones((NQ, NB), dtype=np.int64)
        for Q in range(NQ):
            caps[:, (Q + 1) * TPQ - nfat:(Q + 1) * TPQ] = 2
        perms = []
        ok = True
        for r in range(NCORES):
            p_ = _pack_rank4(rank_data[r][4], caps)
            if p_ is None:
                ok = False
                break
            perms.append(p_)
        if ok:
            break
        nfat += 2
        if nfat > TPQ:
            raise RuntimeError("packing failed")

    prof = tuple(int(x) for x in caps.reshape(-1))
    L = caps.sum(axis=1)
    lbase = np.concatenate([[0], np.cumsum(L)])[:NQ]
    off = np.zeros((NQ, NB), dtype=np.int64)
    for q in range(NQ):
        off[q] = lbase[q] + np.concatenate([[0], np.cumsum(caps[q])[:-1]])
    tot_ch = int(L.sum())
    new2old = [np.argsort(p_) for p_ in perms]

    selscale = 1.0 / SCL if EX_INT8 else 1.0

    per_rank = []
    for r in range(NCORES):
        rl, cc, vv, qs, _ = rank_data[r]
        rl_new = perms[r][rl]
        blk = rl_new // P
        clsmod = rl_new % P

        # gathered source row, relative to its quarter's ag tensor
        r2 = cc // CLOC
        lnew = np.empty(len(cc), dtype=np.int64)
        for r3 in range(NCORES):
            m3 = r2 == r3
            lnew[m3] = perms[r3][cc[m3] % CLOC]
        t2 = lnew // P
        assert (t2 // TPQ == qs).all()
        tq = t2 % TPQ
        p2 = lnew % P
        rowh = r2 * (TPQ * P) + p2 * TPQ + tq

        sel_host = np.zeros((P, tot_ch * P), dtype=EX_NP)
        gidx_flat = np.zeros(tot_ch * P, dtype=np.int64)
        orderkey = np.lexsort((rowh, blk + NB * qs))
        bl_s = blk[orderkey]
        qs_s = qs[orderkey]
        vv_s = vv[orderkey]
        cm_s = clsmod[orderkey]
        rh_s = rowh[orderkey]
        key = qs_s * NB + bl_s
        counts = np.bincount(key, minlength=NQ * NB)
        starts = np.zeros(NQ * NB, dtype=np.int64)
        starts[1:] = np.cumsum(counts)[:-1]
        pos = np.arange(len(key)) - starts[key]
        chlim = caps.reshape(-1)
        assert (pos < chlim[key] * P).all(), "chunk capacity overflow"
        gch = off[qs_s, bl_s] + pos // P
        slot = pos % P
        sel_host[slot, gch * P + cm_s] = (vv_s * selscale).astype(EX_NP)
        gidx_flat[gch * P + slot] = rh_s
        gidxs_host = _wrap_idx16(gidx_flat)

        rows = slice(r * CLOC, (r + 1) * CLOC)
        wr = wt2_w[rows][new2old[r]]
        wt_host = np.ascontiguousarray(
            wr.reshape(NB, P, ND, P).transpose(0, 3, 2, 1)
        ).reshape(NB, P, D).astype(ml_dtypes.bfloat16)
        bias_host = np.ascontiguousarray(
            wt2_b[rows][new2old[r]].reshape(NB, P).T)
        rd = {
            "enc": enc_flat,
            "tgidx": tgidx_host,
            "wt": wt_host,
            "biasv": bias_host,
            "sel": sel_host,
            "gidxs": gidxs_host,
        }
        if ZP_MODE:
            selv = (vv * selscale).astype(EX_NP).astype(np.float64)
            rowsum = np.bincount(rl_new, weights=selv, minlength=CLOC)
            corr = (ZP_EFF * rowsum).astype(np.float32)
            rd["corrv"] = np.ascontiguousarray(corr.reshape(NB, P).T)
        per_rank.append(rd)
    return per_rank, prof, new2old


def kernel(**inputs) -> np.ndarray:
    per_rank, prof, new2old = _prep_host(
        inputs["enc_out"], inputs["wt2_w"], inputs["wt2_b"],
        inputs["A_values"], inputs["batch_idx"], inputs["tgt"],
        inputs["A_indices"])
    nc = _build_program(prof)
    res = None
    last_exc = None
    for _attempt in range(3):
        try:
            res = run_bass_kernel_spmd(
                nc, per_rank, core_ids=list(range(NCORES)), trace=TRACE,
                tmpdir=TRACE_DIR)
            break
        except Exception as e:
            last_exc = e
    if res is None:
        raise last_exc
    global LAST_RESULTS
    LAST_RESULTS = res
    outT_full = np.empty((C, N), dtype=np.float32)
    for r in range(NCORES):
        arr = np.asarray(res.results[r]["outT"])
        arr = arr.reshape(P, NB, N).transpose(1, 0, 2).reshape(CLOC, N)
        outT_full[r * CLOC + new2old[r]] = arr.astype(np.float32)
    return np.ascontiguousarray(outT_full.T)
